# revision 27
# baseline (speedup 1.0000x reference)
"""MoE audio projector kernel for 8 Trainium2 NeuronCores.

Strategy (expert-parallel, host-side routing):
  - Host: pool (K=2), RMSNorm, router softmax + top-4, combine weights.
  - Each of the 8 cores runs one routed expert's SwiGLU over the tokens that
    selected it (gathered + padded to a fixed capacity), plus 1/8 of the
    tokens through the shared expert.
  - Host: scatter-add combine of routed outputs, add shared, final RMSNorm,
    clip, plus the aux loss (router stats only, negligible FLOPs).

Device matmuls run as float32r (tf32-like, full PE rate). Layout: the first
matmul is feature-major (lhsT = w12 as stored in HBM, rhs = x transposed on
host), SwiGLU fused on ScalarE/VectorE, second matmul token-major
(lhsT = activation tile, rhs = w3 as stored), then a per-partition
combine-weight scale.
"""

import numpy as np

E = 8
TOP_K = 4
SCALE = 1.0
EPS = 1e-6
KPOOL = 2
IN_DIM = 2560
OUT_DIM = 2048
RH = 1024
SH = 1024
NCORES = 8
TS = 768          # shared-expert tokens per core (ceil(6000/8) -> 768 padded)
MM_DTYPE = "bf16"  # "bf16" or "f32r" matmul precision

_compiled_cache = {}


def _build_device_kernel(C, mm_dtype=None, loop_n=1):
    import concourse.bacc as bacc
    import concourse.mybir as mybir
    import concourse.tile as tile

    if mm_dtype is None:
        mm_dtype = MM_DTYPE
    MMDT = mybir.dt.bfloat16 if mm_dtype == "bf16" else mybir.dt.float32r
    TMAX = 1152 if mm_dtype == "bf16" else 896
    F32 = mybir.dt.float32
    Silu = mybir.ActivationFunctionType.Silu
    Copy = mybir.ActivationFunctionType.Copy

    K1 = IN_DIM // 128     # 20 contraction tiles for matmul 1
    K2 = RH // 128         # 8 contraction tiles for matmul 2
    G = RH // 128          # 8 gate/val row-tile pairs
    NO = OUT_DIM // 512    # 4 output column blocks

    nc = bacc.Bacc("TRN2", target_bir_lowering=False, debug=False,
                   num_devices=NCORES)

    CB = C // 128
    xgT = nc.dram_tensor("xgT", [IN_DIM, C], MMDT, kind="ExternalInput")
    # combine weights pre-swizzled on host: cw[p, j] = weight of token j*128+p
    cw = nc.dram_tensor("cw", [128, CB], F32, kind="ExternalInput")
    w12 = nc.dram_tensor("w12", [IN_DIM, 2 * RH], MMDT, kind="ExternalInput")
    w3 = nc.dram_tensor("w3", [RH, OUT_DIM], MMDT, kind="ExternalInput")
    xsT = nc.dram_tensor("xsT", [IN_DIM, TS], MMDT, kind="ExternalInput")
    sw12 = nc.dram_tensor("sw12", [IN_DIM, 2 * SH], MMDT, kind="ExternalInput")
    sw3 = nc.dram_tensor("sw3", [SH, OUT_DIM], MMDT, kind="ExternalInput")
    ro = nc.dram_tensor("ro", [C, OUT_DIM], F32, kind="ExternalOutput")
    so = nc.dram_tensor("so", [TS, OUT_DIM], F32, kind="ExternalOutput")

    def chunk_sizes(n):
        # balanced chunks, multiples of 128, each <= TMAX
        nch = -(-n // TMAX)
        base = (n // nch) // 128 * 128
        sizes = [base] * nch
        rem = (n - base * nch) // 128
        for i in range(rem):
            sizes[i] += 128
        assert sum(sizes) == n and all(256 <= s <= TMAX for s in sizes)
        return sizes

    def nt_pieces(s):
        # split a chunk into moving-dim pieces, each >= 256 (f32r full rate)
        out = []
        while s > 512:
            if s - 512 >= 256:
                out.append(512)
                s -= 512
            else:
                out.append(s - 256)
                s = 256
        out.append(s)
        return out

    with tile.TileContext(nc) as tc:
        with (
            tc.tile_pool(name="xch", bufs=1) as xpool,
            tc.tile_pool(name="ach", bufs=1) as apool,
            tc.tile_pool(name="wt", bufs=4 if mm_dtype == "bf16" else 3) as wpool,
            tc.tile_pool(name="w3b", bufs=3 if mm_dtype == "bf16" else 2) as w3pool,
            tc.tile_pool(name="sil", bufs=2) as spool,
            tc.tile_pool(name="ot", bufs=2) as opool,
            tc.tile_pool(name="cwt", bufs=1) as cwpool,
            tc.tile_pool(name="ps", bufs=2, space="PSUM") as pspool,
        ):
            def swiglu_phase(xT_d, w12_d, w3_d, out_d, ntok, cw_d):
                # combine-weight tile is loaded lazily right before the first
                # second-matmul block, so it doesn't delay the critical
                # startup DMAs
                cw_state = {}

                def get_cw():
                    if "t" not in cw_state:
                        t = cwpool.tile([128, C // 128], F32, tag="cwa")
                        nc.sync.dma_start(t[:], cw_d[:])
                        cw_state["t"] = t
                    return cw_state["t"]
                xT_r = xT_d[:].rearrange("(ko ki) t -> ki ko t", ki=128)
                w12_r = w12_d[:].rearrange("(ko ki) m -> ki ko m", ki=128)
                w3_r = w3_d[:].rearrange("(ko ki) m -> ki ko m", ki=128)
                WSTEP = 5

                def load_w12(mh):
                    # split into k-piece DMAs: matmuls can start on the first
                    # piece, and pieces spread across DMA queues
                    wg = wpool.tile([128, K1, 128], MMDT, tag="wt")
                    wv = wpool.tile([128, K1, 128], MMDT, tag="wt")
                    for ks in range(0, K1, WSTEP):
                        nc.sync.dma_start(
                            wg[:, ks:ks + WSTEP, :],
                            w12_r[:, ks:ks + WSTEP, mh * 128:(mh + 1) * 128])
                        nc.sync.dma_start(
                            wv[:, ks:ks + WSTEP, :],
                            w12_r[:, ks:ks + WSTEP,
                                  (G + mh) * 128:(G + mh + 1) * 128])
                    return wg, wv

                c0 = 0
                for tcn in chunk_sizes(ntok):
                    # weights for the first gate/val pair before the x chunk,
                    # so the PE can start as soon as the first k-slices land
                    preload = load_w12(0)
                    xch = xpool.tile([128, K1, tcn], MMDT, tag="xch")
                    for k in range(K1):
                        nc.sync.dma_start(xch[:, k, :], xT_r[:, k, c0:c0 + tcn])
                    ach = apool.tile([128, G, tcn], MMDT, tag="ach")
                    for mh in range(G):
                        wg, wv = preload if mh == 0 else load_w12(mh)
                        nt0 = 0
                        for ntw in nt_pieces(tcn):
                            pg = pspool.tile([128, ntw], F32, tag="pg")
                            pv = pspool.tile([128, ntw], F32, tag="pv")
                            for k in range(K1):
                                nc.tensor.matmul(
                                    pg[:], wg[:, k, :], xch[:, k, nt0:nt0 + ntw],
                                    start=(k == 0), stop=(k == K1 - 1))
                            for k in range(K1):
                                nc.tensor.matmul(
                                    pv[:], wv[:, k, :], xch[:, k, nt0:nt0 + ntw],
                                    start=(k == 0), stop=(k == K1 - 1))
                            sil = spool.tile([128, ntw], F32, tag="sil")
                            nc.scalar.activation(sil[:], pg[:], Silu)
                            nc.vector.tensor_mul(
                                ach[:, mh, nt0:nt0 + ntw], sil[:], pv[:])
                            nt0 += ntw
                    OW = 512
                    for nb in range(OUT_DIM // OW):
                        w3b = w3pool.tile([128, K2, OW], MMDT, tag="w3b")
                        for ks in range(0, K2, 2):
                            nc.sync.dma_start(
                                w3b[:, ks:ks + 2, :],
                                w3_r[:, ks:ks + 2, nb * OW:(nb + 1) * OW])
                        for t0 in range(0, tcn, 128):
                            ot = opool.tile([128, OW], F32, tag="ot")
                            for sub in range(OW // 512):
                                po = pspool.tile([128, 512], F32, tag="po")
                                for k2 in range(K2):
                                    nc.tensor.matmul(
                                        po[:], ach[:, k2, t0:t0 + 128],
                                        w3b[:, k2, sub * 512:(sub + 1) * 512],
                                        start=(k2 == 0), stop=(k2 == K2 - 1))
                                osl = ot[:, sub * 512:(sub + 1) * 512]
                                if cw_d is not None:
                                    tidx = (c0 + t0) // 128
                                    nc.vector.tensor_scalar_mul(
                                        osl, po[:], get_cw()[:, tidx:tidx + 1])
                                else:
                                    nc.scalar.activation(osl, po[:], Copy)
                            nc.sync.dma_start(
                                out_d[:][c0 + t0:c0 + t0 + 128,
                                         nb * OW:(nb + 1) * OW], ot[:])
                    c0 += tcn

            def body():
                swiglu_phase(xgT, w12, w3, ro, C, cw)
                swiglu_phase(xsT, sw12, sw3, so, TS, None)

            if loop_n > 1:
                # timing-only mode: repeat the whole computation on-device
                with tc.For_i(0, loop_n, 1):
                    body()
            else:
                body()

    nc.compile()
    return nc


def _get_compiled(C):
    nc = _compiled_cache.get(C)
    if nc is None:
        nc = _build_device_kernel(C)
        _compiled_cache[C] = nc
    return nc


def kernel(x, ln_pre_w, router_w, shared_w12, shared_w3,
           routed_w12, routed_w3, ln_post_w):
    from concourse.bass_utils import run_bass_kernel_spmd

    x = np.asarray(x, dtype=np.float32)
    B, S, D = x.shape
    pad = (-S) % KPOOL
    if pad:
        x = np.pad(x, ((0, 0), (0, pad), (0, 0)))
    nS = (S + pad) // KPOOL
    xt = np.ascontiguousarray(x).reshape(B * nS, D * KPOOL)
    T = xt.shape[0]

    ln_pre_w = np.asarray(ln_pre_w, dtype=np.float32)
    ln_post_w = np.asarray(ln_post_w, dtype=np.float32)
    router_w = np.asarray(router_w, dtype=np.float32)
    shared_w12 = np.ascontiguousarray(np.asarray(shared_w12, dtype=np.float32))
    shared_w3 = np.ascontiguousarray(np.asarray(shared_w3, dtype=np.float32))
    routed_w12 = np.asarray(routed_w12, dtype=np.float32)
    routed_w3 = np.asarray(routed_w3, dtype=np.float32)

    # ---- host: RMSNorm (pre) ----
    var = np.mean(np.square(xt, dtype=np.float64), axis=-1, keepdims=True)
    norm = (xt * (1.0 / np.sqrt(var + EPS))).astype(np.float32) * ln_pre_w

    # ---- host: router softmax / top-k / combine weights / aux loss ----
    logits = (norm @ router_w.T).astype(np.float32)
    z = (logits - logits.max(-1, keepdims=True)).astype(np.float64)
    ez = np.exp(z)
    routing = ez / ez.sum(-1, keepdims=True)          # float64 [T, E]
    imp = routing.sum(axis=0)
    aux_loss = np.float32(np.sum(imp * imp) / (T * T) * E)

    order = np.argsort(-routing, axis=-1, kind="stable")
    topi = order[:, :TOP_K]
    topw = np.take_along_axis(routing, topi, axis=-1)
    topw = topw / (topw.sum(-1, keepdims=True) + 1e-20) * SCALE
    combine = np.zeros((T, E), np.float32)
    np.put_along_axis(combine, topi, topw.astype(np.float32), axis=-1)

    # ---- host: per-expert gather ----
    member = np.zeros((T, E), bool)
    np.put_along_axis(member, topi, True, axis=-1)
    idx = [np.nonzero(member[:, e])[0] for e in range(E)]
    counts = np.array([len(i) for i in idx])
    C = int(np.ceil(max(counts.max(), 256) / 128.0) * 128)

    if MM_DTYPE == "bf16":
        import ml_dtypes
        mmdt_np = np.dtype(ml_dtypes.bfloat16)
    else:
        mmdt_np = np.dtype(np.float32)

    normT = np.ascontiguousarray(norm.T).astype(mmdt_np)   # [IN_DIM, T]
    in_maps = []
    Tpad = NCORES * TS
    norm_pad = np.zeros((Tpad, IN_DIM), mmdt_np)
    norm_pad[:T] = norm
    xsT_all = np.ascontiguousarray(
        norm_pad.reshape(NCORES, TS, IN_DIM).transpose(0, 2, 1))

    sw12_mm = shared_w12.astype(mmdt_np)
    sw3_mm = shared_w3.astype(mmdt_np)
    for e in range(E):
        xgT = np.zeros((IN_DIM, C), mmdt_np)
        xgT[:, :counts[e]] = normT[:, idx[e]]
        cw_flat = np.zeros(C, np.float32)
        cw_flat[:counts[e]] = combine[idx[e], e]
        # device expects cw[p, j] = weight of token j*128+p
        cw = np.ascontiguousarray(cw_flat.reshape(C // 128, 128).T)
        in_maps.append({
            "xgT": xgT,
            "cw": cw,
            "w12": np.ascontiguousarray(routed_w12[e]).astype(mmdt_np),
            "w3": np.ascontiguousarray(routed_w3[e]).astype(mmdt_np),
            "xsT": xsT_all[e],
            "sw12": sw12_mm,
            "sw3": sw3_mm,
        })

    nc = _get_compiled(C)
    results = run_bass_kernel_spmd(
        nc, in_maps, core_ids=list(range(NCORES))).results

    # ---- host: combine ----
    routed = np.zeros((T, OUT_DIM), np.float32)
    for e in range(E):
        routed[idx[e]] += results[e]["ro"][:counts[e]]
    shared = np.concatenate([results[d]["so"] for d in range(NCORES)])[:T]

    y = shared + routed
    var2 = np.mean(np.square(y, dtype=np.float64), axis=-1, keepdims=True)
    final = (y * (1.0 / np.sqrt(var2 + EPS))).astype(np.float32) * ln_post_w
    final = np.clip(final, -30.0, 30.0).reshape(B, nS, OUT_DIM)
    return final, aux_loss
